# revision 4
# baseline (speedup 1.0000x reference)
"""Trainium2 Bass kernel: 5-tap Kaiser circular filter along H then W of a
(16, 3, 1024, 1024) fp32 tensor. Data-parallel over batch across 8 cores.

Per core: 2 batches x 3 channels = 6 independent (1024, 1024) slices.

Per slice, split H into 9 output blocks (8 x 124 rows + 1 x 32 rows). For
each block:
  - DMA the 128-row (36 for the last block) input window rows
    [124j-2, 124j+126) mod 1024 from HBM (halo load, circular wrap handled
    with a second small DMA).
  - H-filter as a banded matmul on the tensor engine: stationary A[k, m] =
    kernel[m+4-k] ([128, 124] band), moving operand = the input window,
    PSUM out = [124, 1024] (two N=512 matmuls, full fp32).
  - Copy PSUM -> SBUF with a 2-column circular halo on each side (scalar
    engine), giving ext [124, 1028].
  - W-filter as 5 free-dim-shifted multiply-accumulates on the vector
    engine (1 tensor_scalar + 4 scalar_tensor_tensor, fp32).
  - DMA the [124, 1024] result block back to HBM.
"""

import numpy as np

B, C, H, W = 16, 3, 1024, 1024
N_CORES = 8
S = (B // N_CORES) * C  # slices per core
TAPS = 5
HALO = TAPS // 2  # 2
STRIDE = 124  # output rows per full block (128 - 2*HALO)
NBLK = 9  # 8 full blocks + one 32-row tail
TAIL_M = H - 8 * STRIDE  # 32

_cache = {}


def _build_with_taps(kk, repeat=1):
    """kk: numpy [5] float32 tap weights. Returns compiled Bass object."""
    import concourse.bacc as bacc
    import concourse.mybir as mybir
    import concourse.tile as tile

    f32 = mybir.dt.float32
    nc = bacc.Bacc("TRN2", target_bir_lowering=False, debug=False, num_devices=N_CORES)

    x_d = nc.dram_tensor("x", [S, H, W], f32, kind="ExternalInput")
    a_d = nc.dram_tensor("afilt", [128, STRIDE], f32, kind="ExternalInput")
    y_d = nc.dram_tensor("y", [S, H, W], f32, kind="ExternalOutput")

    kk = [float(v) for v in kk]

    with tile.TileContext(nc) as tc:
        with (
            tc.tile_pool(name="wpool", bufs=1) as wpool,
            tc.tile_pool(name="inp", bufs=3) as inp,
            tc.tile_pool(name="psum", bufs=2, space="PSUM") as psum,
            tc.tile_pool(name="extp", bufs=3) as extp,
            tc.tile_pool(name="outp", bufs=3) as outp,
        ):
            a_s = wpool.tile([128, STRIDE], f32)
            nc.sync.dma_start(a_s[:], a_d[:])

            for _ in range(repeat):
                for s in range(S):
                    for j in range(NBLK):
                        m = STRIDE if j < 8 else TAIL_M  # output rows
                        kdim = m + 2 * HALO  # input rows (128 or 36)
                        obase = j * STRIDE

                        in_t = inp.tile([128, W], f32)
                        # input window: virtual rows [obase-2, obase-2+kdim)
                        r0 = obase - HALO
                        if r0 < 0:
                            # wrap at the top: rows [H+r0, H) then [0, kdim+r0)
                            nc.sync.dma_start(
                                in_t[0:-r0, :], x_d[s, H + r0 : H, :]
                            )
                            nc.sync.dma_start(
                                in_t[-r0:kdim, :], x_d[s, 0 : kdim + r0, :]
                            )
                        elif r0 + kdim > H:
                            # wrap at the bottom
                            n1 = H - r0
                            nc.sync.dma_start(in_t[0:n1, :], x_d[s, r0:H, :])
                            nc.sync.dma_start(
                                in_t[n1:kdim, :], x_d[s, 0 : kdim - n1, :]
                            )
                        else:
                            nc.sync.dma_start(in_t[0:kdim, :], x_d[s, r0 : r0 + kdim, :])

                        ps = psum.tile([STRIDE, W], f32)
                        for half in range(0, W, 512):
                            nc.tensor.matmul(
                                ps[0:m, half : half + 512],
                                a_s[0:kdim, 0:m],
                                in_t[0:kdim, half : half + 512],
                                start=True,
                                stop=True,
                            )

                        ext = extp.tile([STRIDE, W + 4], f32)
                        nc.scalar.copy(ext[0:m, 2 : 2 + W], ps[0:m, :])
                        nc.scalar.copy(ext[0:m, 0:2], ps[0:m, W - 2 : W])
                        nc.scalar.copy(ext[0:m, 2 + W : 4 + W], ps[0:m, 0:2])

                        out_t = outp.tile([STRIDE, W], f32)
                        # out[p, w] = sum_d kk[d] * ext[p, w + 4 - d]
                        nc.vector.tensor_scalar_mul(
                            out_t[0:m, :], ext[0:m, 4 : 4 + W], kk[0]
                        )
                        for d in range(1, TAPS):
                            sft = 4 - d
                            nc.vector.scalar_tensor_tensor(
                                out_t[0:m, :],
                                ext[0:m, sft : sft + W],
                                kk[d],
                                out_t[0:m, :],
                                mybir.AluOpType.mult,
                                mybir.AluOpType.add,
                            )

                        nc.sync.dma_start(y_d[s, obase : obase + m, :], out_t[0:m, :])

    nc.compile()
    return nc


def _afilt_from_taps(kk):
    a = np.zeros((128, STRIDE), dtype=np.float32)
    for mcol in range(STRIDE):
        for d in range(TAPS):
            k = mcol + 4 - d
            if 0 <= k < 128:
                a[k, mcol] = kk[d]
    return a


def kernel(x, kernel):
    from concourse.bass_utils import run_bass_kernel_spmd

    x = np.asarray(x, dtype=np.float32)
    kk = np.asarray(kernel, dtype=np.float32).reshape(-1)
    assert x.shape == (B, C, H, W)
    assert kk.shape == (TAPS,)

    key = kk.tobytes()
    if key not in _cache:
        _cache[key] = _build_with_taps(kk)
    nc = _cache[key]

    afilt = _afilt_from_taps(kk)
    per_core = B // N_CORES
    in_maps = []
    for i in range(N_CORES):
        shard = np.ascontiguousarray(
            x[i * per_core : (i + 1) * per_core].reshape(S, H, W)
        )
        in_maps.append({"x": shard, "afilt": afilt})

    res = run_bass_kernel_spmd(nc, in_maps, core_ids=list(range(N_CORES)))
    out = np.empty((B, C, H, W), dtype=np.float32)
    for i in range(N_CORES):
        out[i * per_core : (i + 1) * per_core] = res.results[i]["y"].reshape(
            per_core, C, H, W
        )
    return out
